# revision 21
# baseline (speedup 1.0000x reference)
"""AnomalyAttention Trainium2 kernel (8 NeuronCores, sequence-sharded).

Reference computation (B=1, L=8192, D=64):
  prior[i,j]  = inv_norm_i * exp(-(i-j)^2 / (2 s_i^2)) / row_sum[j]
                  (row_sum[j] = sum_k inv_norm_j * exp(-(j-k)^2 / (2 s_j^2));
                   the divisor indexes the COLUMN j -- faithful to the torch code)
  series      = softmax(Q K^T / sqrt(dk), axis=-1)
  out         = series @ V

Sharding: core m owns rows [1024 m, 1024 m + 1024). K, V, Sigma replicated.

Device-side structure per core:
  * prior: with sigma in [0.5, 2), exp(-d^2/(2 s^2)) underflows f32 to 0 for
    |d| >= 27, so the prior is a banded matrix.  Each 128-row tile computes a
    [128, 192] window around the diagonal; everything else is exactly 0 and is
    materialized host-side (the device output buffers are pre-zeroed anyway).
  * series: S^T tiles (KQ^T) -> exp -> PV matmul with a ones-column appended to
    V (gives the softmax denominator l_i for free), then a second QK^T pass:
    exp(S/8) then a per-partition multiply by 1/l_i, streamed to DRAM in
    1536-column slabs.  Softmax max-subtraction is skipped: |S/8| <= ~6 so
    exp cannot overflow.
  * matmuls run as float32r (full-rate fp32 PE mode; plain fp32 is 4x slower).
"""

import numpy as np

import concourse.bacc as bacc
import concourse.mybir as mybir
import concourse.tile as tile
from concourse import bass_utils


def _pin_act_tables():
    """Make the act-table pass choose natural_log_exp_and_others for both
    Exp and Ln so the kernel needs exactly one table load.  Set ids must
    stay stable, so we only *remove* claims from other sets."""
    import concourse.hw_specs as hw_specs
    orig = hw_specs.get_activation_tables
    exp_ln = {mybir.ActivationFunctionType.Exp, mybir.ActivationFunctionType.Ln}

    def patched(arch):
        t = orig(arch)
        out = {}
        for name, fns in t.items():
            if name != "natural_log_exp_and_others":
                fns = set(fns) - exp_ln
            out[name] = fns
        return out

    hw_specs.get_activation_tables = patched
    bacc.get_activation_tables = patched


_pin_act_tables()

L = 8192
D = 64
N_CORES = 8
RPC = L // N_CORES        # 1024 rows per core
NT = RPC // 128           # 8 row-tiles per core
W = 32                    # band half-width
BAND = 128 + 2 * W        # 192-column band window per row-tile
ICW = 256                 # i-chunk width for stage 1
NICH = RPC // ICW         # 4 i-chunks
NJT = L // 128            # 64 j-tiles
SCALE = 0.125             # 1/sqrt(dk)
INV_SQRT_2PI = 0.3989422804014327
WIN = 1152                # row_sum window width (9 * 128)
CW = 1536                 # PSUM working-chunk width (3 banks)

f32 = mybir.dt.float32
f32r = mybir.dt.float32r
EXP = mybir.ActivationFunctionType.Exp
LOG = mybir.ActivationFunctionType.Ln


def build_nc(reps=1, stagger=False):
    """reps>1 wraps the whole body in a Tile For_i loop -- used only for
    timing (the NEFF re-runs the identical computation reps times)."""
    nc = bacc.Bacc("TRN2", target_bir_lowering=False, debug=False,
                   num_devices=N_CORES)

    qt = nc.declare_dram_parameter("qt", [64, RPC], f32r, isOutput=False)
    kt = nc.declare_dram_parameter("kt", [64, L], f32r, isOutput=False)
    vo = nc.declare_dram_parameter("vo", [128, NJT * 65], f32r, isOutput=False)
    PPACK = NT + 9 + 9 * 65 + BAND + 128
    ppack = nc.declare_dram_parameter("ppack", [128, PPACK], f32, isOutput=False)

    band_o = nc.declare_dram_parameter("band_o", [NT, 128, BAND], f32, isOutput=True)
    ser_o = nc.declare_dram_parameter("ser_o", [RPC, L], f32, isOutput=True)
    out_o = nc.declare_dram_parameter("out_o", [RPC, 64], f32, isOutput=True)

    HALF = NJT // 2
    with tile.TileContext(nc) as tc:
        with (
            tc.tile_pool(name="const", bufs=1) as cp,
            tc.tile_pool(name="ser", bufs=3) as serp,
            tc.tile_pool(name="pt", bufs=4) as ptp,
            tc.tile_pool(name="sm", bufs=2) as smp,
            tc.tile_pool(name="stps", bufs=2, space="PSUM") as stps,
            tc.tile_pool(name="otp", bufs=2, space="PSUM") as otp,
        ):
          from contextlib import nullcontext
          import concourse.mybir as _mb
          loop_cm = (tc.For_i(0, reps, 1, staggered_reset=stagger,
                              hint_engines=(_mb.EngineType.PE,
                                            _mb.EngineType.Activation,
                                            _mb.EngineType.DVE,
                                            _mb.EngineType.SP))
                     if reps > 1 else nullcontext())
          with loop_cm:
            # ---- inputs: attention-critical loads on the HWDGE queue --------
            # small prior-path inputs in ONE dma FIRST (tiny; unblocks the
            # rs/band chain that the scheduler fronts on the ACT queue)
            pk = cp.tile([128, PPACK], f32, tag="pk")
            nc.sync.dma_start(pk[:], ppack[:])
            o0 = 0
            srt = pk[:, o0:o0 + NT]; o0 += NT
            swin = pk[:, o0:o0 + 9]; o0 += 9
            nd2 = pk[:, o0:o0 + 9 * 65]; o0 += 9 * 65
            d2b = pk[:, o0:o0 + BAND]; o0 += BAND
            id_sb = pk[:, o0:o0 + 128]

            qt_sb = cp.tile([64, RPC], f32r, tag="qt")
            NQ = 4
            KQW = L // NQ                  # kt quarter width (j cols)
            VQT = NJT // NQ                # vo j-tiles per quarter
            kt_q = [cp.tile([64, KQW], f32r, tag=f"kt{q}", name=f"ktq{q}") for q in range(NQ)]
            vo_q = [cp.tile([128, VQT * 65], f32r, tag=f"vo{q}", name=f"voq{q}") for q in range(NQ)]
            nc.sync.dma_start(qt_sb[:], qt[:])
            for q in range(NQ):
                nc.sync.dma_start(kt_q[q][:], kt[:, KQW * q:KQW * (q + 1)])
                nc.sync.dma_start(vo_q[q][:], vo[:, VQT * 65 * q:VQT * 65 * (q + 1)])

            def kt_ap(j0, w):
                q, off = divmod(j0, KQW)
                return kt_q[q][:, off:off + w]

            def vo_ap(jt):
                q, r = divmod(jt, VQT)
                return vo_q[q][:, 65 * r:65 * r + 65]

            # ---- sigma prep (per-row-tile scale/scalar vectors) -------------
            i2s = cp.tile([128, NT], f32, tag="i2s")      # -1/(2 s^2)
            invn = cp.tile([128, NT], f32, tag="invn")    # inv_norm
            tmp = cp.tile([128, NT], f32, tag="tmpA")
            nc.vector.tensor_tensor(tmp[:], srt, srt, op=mybir.AluOpType.mult)
            nc.vector.tensor_scalar_mul(tmp[:], tmp[:], -2.0)
            nc.vector.reciprocal(i2s[:], tmp[:])
            nc.vector.reciprocal(tmp[:], srt)
            nc.vector.tensor_scalar_mul(invn[:], tmp[:], INV_SQRT_2PI)

            outv = cp.tile([128, NT * 64], f32, tag="outv")

            def rs_window():
                """row_sum over the banded window -> 1/rs [128, 9] col-major."""
                wtmp = cp.tile([128, 9], f32, tag="wtmp")
                wi2s = cp.tile([128, 9], f32, tag="wi2s")
                nc.vector.tensor_tensor(wtmp[:], swin, swin, op=mybir.AluOpType.mult)
                nc.vector.tensor_scalar_mul(wtmp[:], wtmp[:], 2.0)
                nc.vector.reciprocal(wi2s[:], wtmp[:])
                ex = cp.tile([128, 9 * 65], f32, tag="ex")
                nc.vector.tensor_tensor(
                    ex[:].rearrange("p (a b) -> p a b", a=9),
                    nd2.rearrange("p (a b) -> p a b", a=9),
                    wi2s[:].unsqueeze(2).broadcast_to([128, 9, 65]),
                    op=mybir.AluOpType.mult)
                nc.scalar.activation(ex[:], ex[:], EXP)
                rs = cp.tile([128, 9], f32, tag="rs")
                nc.vector.tensor_reduce(rs[:], ex[:].rearrange("p (a b) -> p a b", a=9),
                                        axis=mybir.AxisListType.X, op=mybir.AluOpType.add)
                nc.vector.reciprocal(wtmp[:], swin)
                nc.vector.tensor_scalar_mul(wtmp[:], wtmp[:], INV_SQRT_2PI)
                nc.vector.tensor_tensor(rs[:], rs[:], wtmp[:], op=mybir.AluOpType.mult)
                rrs_w = cp.tile([128, 9], f32, tag="rrsw")
                nc.vector.reciprocal(rrs_w[:], rs[:])
                return rrs_w

            def stage1(ic):
                """KQ^T -> exp -> (Vo^T P) accumulation; returns ot psum tile."""
                ot = otp.tile([65, ICW], f32, tag="m")
                gw = CW // ICW
                jt0 = 0
                while jt0 < NJT:
                    g = min(gw, NJT - jt0)
                    st = stps.tile([128, CW], f32, tag="st")
                    for b in range(g):
                        jt = jt0 + b
                        nc.tensor.matmul(
                            st[:, ICW * b:ICW * b + ICW],
                            kt_ap(128 * jt, 128),
                            qt_sb[:, ICW * ic:ICW * ic + ICW],
                            start=True, stop=True)
                    pt = ptp.tile([128, CW], f32r, tag="pt")
                    nc.scalar.activation(pt[:, 0:ICW * g], st[:, 0:ICW * g],
                                         EXP, scale=SCALE)
                    for b in range(g):
                        jt = jt0 + b
                        nc.tensor.matmul(
                            ot[:], vo_ap(jt), pt[:, ICW * b:ICW * b + ICW],
                            start=(jt == 0), stop=(jt == NJT - 1))
                    jt0 += g
                return ot

            def rrs_broadcast(rrs_w):
                """rrs window -> [128, WIN] partition-broadcast (PSUM via otp)."""
                rtr = otp.tile([9, 128], f32, tag="m")
                nc.tensor.transpose(rtr[:], rrs_w[:], id_sb)
                rtr_sb = cp.tile([9, 128], f32, tag="rtrsb")
                nc.vector.tensor_copy(rtr_sb[:], rtr[:])
                rrow = cp.tile([1, WIN], f32, tag="rrow")
                for c in range(9):
                    nc.gpsimd.dma_start(rrow[0:1, 128 * c:128 * c + 128],
                                        rtr_sb[c:c + 1, :])
                ones = cp.tile([1, 128], f32, tag="ones")
                nc.vector.memset(ones[:], 1.0)
                rbc = cp.tile([128, WIN], f32, tag="rbc")
                for k in range(3):
                    w = 512 if k < 2 else 128
                    bc = otp.tile([128, 512], f32, tag="m")
                    nc.tensor.matmul(bc[:, 0:w], ones[:, :],
                                     rrow[0:1, 512 * k:512 * k + w],
                                     start=True, stop=True)
                    nc.vector.tensor_copy(rbc[:, 512 * k:512 * k + w], bc[:, 0:w])
                return rbc

            def prior_band(rbc):
                band_all = cp.tile([128, NT * BAND], f32, tag="bandall")
                for t in range(NT):
                    kw = smp.tile([128, BAND], f32, tag="kw")
                    nc.scalar.activation(kw[:], d2b, EXP, scale=i2s[:, t:t + 1])
                    nc.vector.scalar_tensor_tensor(
                        band_all[:, BAND * t:BAND * (t + 1)], kw[:],
                        invn[:, t:t + 1],
                        rbc[:, 128 * t + 32:128 * t + 32 + BAND],
                        op0=mybir.AluOpType.mult, op1=mybir.AluOpType.mult)
                nc.gpsimd.dma_start(
                    band_o[:].rearrange("t p c -> p t c"),
                    band_all[:].rearrange("p (t c) -> p t c", t=NT))

            def ell_proc(ic, ot):
                """Extract l_i and out rows; produce 1/l per-partition."""
                otsb = smp.tile([65, ICW], f32, tag="otsb")
                nc.vector.tensor_copy(otsb[:], ot[:])
                nb = ICW // 128
                rl = smp.tile([128, nb], f32, tag="rl")
                for b in range(nb):
                    it = nb * ic + b
                    tr = otp.tile([128, 65], f32, tag="m")
                    nc.tensor.transpose(tr[:], otsb[:, 128 * b:128 * b + 128],
                                        id_sb[0:65, 0:65])
                    nc.vector.reciprocal(rl[:, b:b + 1], tr[:, 64:65])
                    nc.vector.tensor_scalar(
                        outv[:, 64 * it:64 * it + 64], tr[:, 0:64],
                        rl[:, b:b + 1], None, op0=mybir.AluOpType.mult)
                return rl

            def stage3(ic, rl):
                """QK^T -> exp(S/8) -> *(1/l) -> series rows (chunk slabs)."""
                nb = ICW // 128
                for b in range(nb):
                    it = nb * ic + b
                    ser = serp.tile([128, L], f32, tag="ser")
                    j0 = 0
                    while j0 < L:
                        w = min(CW, L - j0)
                        s3 = stps.tile([128, CW], f32, tag="st")
                        for h in range(w // 512):
                            nc.tensor.matmul(
                                s3[:, 512 * h:512 * h + 512],
                                qt_sb[:, 128 * it:128 * it + 128],
                                kt_ap(j0 + 512 * h, 512), start=True, stop=True)
                        nc.scalar.activation(
                            ser[:, j0:j0 + w], s3[:, 0:w], EXP, scale=SCALE)
                        nc.vector.tensor_scalar(
                            ser[:, j0:j0 + w], ser[:, j0:j0 + w],
                            rl[:, b:b + 1], None, op0=mybir.AluOpType.mult)
                        nc.sync.dma_start(
                            ser_o[128 * it:128 * it + 128, j0:j0 + w],
                            ser[:, j0:j0 + w])
                        j0 += w

            for ic in range(NICH):
                ot = stage1(ic)
                if ic == 0:
                    rbc = rrs_broadcast(rs_window())
                    prior_band(rbc)
                rl = ell_proc(ic, ot)
                stage3(ic, rl)

            nc.sync.dma_start(
                out_o[:].rearrange("(t p) d -> p t d", p=128),
                outv[:].rearrange("p (t d) -> p t d", t=NT))

    nc.compile()
    return nc


def make_in_maps(Sigma, Q, K, V):
    """Sigma [1,L,1], Q/K/V [1,L,D] float32 -> per-core input dicts."""
    Sig = np.ascontiguousarray(Sigma.reshape(L).astype(np.float32))
    Qf = np.ascontiguousarray(Q.reshape(L, D).astype(np.float32))
    Kf = np.ascontiguousarray(K.reshape(L, D).astype(np.float32))
    Vf = np.ascontiguousarray(V.reshape(L, D).astype(np.float32))

    kt = np.ascontiguousarray(Kf.T)                       # [64, L]
    voc = np.concatenate([Vf, np.ones((L, 1), np.float32)], axis=1)  # [L, 65]
    # pack Vo into [128, NJT*65]: vo[p, 65*jt+c] = Vo[128*jt+p, c]
    vo = np.ascontiguousarray(
        voc.reshape(NJT, 128, 65).transpose(1, 0, 2).reshape(128, NJT * 65))
    ident = np.eye(128, dtype=np.float32)
    # band distance constant: D2[p, c] = (p + 32 - c)^2
    p_i = np.arange(128)[:, None]
    c_i = np.arange(BAND)[None, :]
    d2band = ((p_i + W - c_i).astype(np.float32)) ** 2

    in_maps = []
    for m in range(N_CORES):
        r0 = RPC * m
        qt = np.ascontiguousarray(Qf[r0:r0 + RPC].T)      # [64, 1024]
        sig_rt = np.ascontiguousarray(
            Sig[r0:r0 + RPC].reshape(NT, 128).T)          # [128, 8]
        j0 = r0 - 64
        jwin = j0 + np.arange(WIN)
        valid = (jwin >= 0) & (jwin < L)
        sw = np.ones(WIN, np.float32)
        sw[valid] = Sig[jwin[valid]]
        sig_win = np.ascontiguousarray(sw.reshape(9, 128).T)  # [128, 9]
        # negd2m[p, 65c+dd] for j = j0+128c+p, d = dd-32:
        #   in-range j and j+d: -(d^2);  out-of-range j: 0 at dd=32 else -1e30;
        #   in-range j, out-of-range j+d: -1e30
        dd = np.arange(65) - W                             # [-32..32]
        jj = jwin.reshape(9, 128).T                        # [128, 9]
        jv = (jj >= 0) & (jj < L)                          # j valid
        tv = (jj[:, :, None] + dd[None, None, :] >= 0) & \
             (jj[:, :, None] + dd[None, None, :] < L)
        neg = np.where(jv[:, :, None] & tv, -(dd.astype(np.float32) ** 2)[None, None, :],
                       np.float32(-1e30))
        ctr = (~jv)                                        # OOR center: unmask d=0
        neg[:, :, W][ctr] = 0.0
        negd2m = np.ascontiguousarray(neg.reshape(128, 9 * 65).astype(np.float32))
        ppack = np.concatenate(
            [sig_rt, sig_win, negd2m, d2band, ident], axis=1)
        in_maps.append({"qt": qt, "kt": kt, "vo": vo,
                        "ppack": np.ascontiguousarray(ppack)})
    return in_maps


def assemble(results):
    """Per-core outputs -> (prior [1,L,L], series [1,L,L], out [1,L,D])."""
    prior = np.zeros((L, L), np.float32)
    series = np.empty((L, L), np.float32)
    out = np.empty((L, D), np.float32)
    for m in range(N_CORES):
        r0 = RPC * m
        series[r0:r0 + RPC] = results[m]["ser_o"]
        out[r0:r0 + RPC] = results[m]["out_o"]
        band = results[m]["band_o"]                        # [NT, 128, BAND]
        for t in range(NT):
            jlo = r0 + 128 * t - W
            cs = max(0, -jlo)
            ce = BAND - max(0, jlo + BAND - L)
            prior[r0 + 128 * t:r0 + 128 * t + 128, jlo + cs:jlo + ce] = \
                band[t][:, cs:ce]
    return prior[None], series[None], out[None]


_CACHE = {}


def kernel(Sigma, Q, K, V, dk):
    if "nc" not in _CACHE:
        _CACHE["nc"] = build_nc()
    nc = _CACHE["nc"]
    in_maps = make_in_maps(np.asarray(Sigma), np.asarray(Q), np.asarray(K),
                           np.asarray(V))
    res = bass_utils.run_bass_kernel_spmd(nc, in_maps, core_ids=list(range(N_CORES)))
    return assemble(res.results)

